# revision 1
# baseline (speedup 1.0000x reference)
"""Trainium2 Bass kernel for nn_Classifier (capsule-style conv + routing).

Math (validated against the jax reference to ~1e-5 rel err):
  W = conv_w[:,0,:]                                   # [16, 640]
  y[b,i,o]   = relu(sum_t x[b,i,t] W[t,o] + conv_b[o])          (conv as matmul, K=16)
  U[b,k,i,d] = y[b,i,k*64+d]
  Usum[b,k,d]= sum_i U[b,k,i,d]
  logits     = (U . Usum)/4            -> stable softmax over i  -> C
  Cb         = C + B_bias[k,i]
  S[b,k,:]   = sum_i Cb[b,k,i] U[b,k,i,:]
  out[b,k]   = (n2/(n2+1)) * (sqrt(n2)/(sqrt(n2)+1e-7)),  n2 = |S|^2

Sharding: data-parallel over batch, 8 batches per core, 8 cores (SPMD).

Per-core layout (b = 4g+j, g in {0,1} bgroups, j in 0..3):
  xT4[g]  [128,512]  rows 32j+t = x[b,i,t], row 32j+16 = 1.0 (bias fold)
  w4      [128,640]  rows 32j+t = W[t,o],  row 32j+16 = conv_b
  yr_oi[b][128,2560] chunk c: [o=c*128+p, i]   (PE conv, row-tiled 4x over j)
  yr_io[b][128,2560] chunk q: [i=q*128+p, o]   (same conv, other orientation)
  Usum via accum_out during yr_oi eviction; logits via block-diag G on PE;
  softmax via reduce_max(negate)+exp(bias=-max, accum=Z); Cb=C/Z+B (one fused op);
  Cb transposed on PE; S = sum_q CbT_q.T @ yr_io_q on PE (col-tiled 4x over j);
  norm + squash scalar tail; sqrt computed as exp(0.5*ln(x)) to stay in one
  ACT table set (relu/exp/ln).
"""

import numpy as np

import concourse.bass as bass
import concourse.mybir as mybir
import concourse.tile as tile
from concourse import bacc
from concourse.bass_utils import run_bass_kernel_spmd

F32 = mybir.dt.float32
F32R = mybir.dt.float32r

B_FULL = 64
N = 512          # num timecaps (routing dim m/i)
DT = 16          # dim timecaps (conv contraction)
K = 10           # classes
D = 64           # dim classes
NO = K * D       # 640 conv output channels
NCORES = 8
BPC = B_FULL // NCORES   # 8 batches per core
EPS = 1e-7

USE_F32R = True          # fp32 data streamed in float32r mode (4x faster PE)
ROUTING_BF16 = False      # bf16 operands for the col-tiled logits/S matmuls
BF16 = mybir.dt.bfloat16


def _build_program():
    nc = bacc.Bacc("TRN2", target_bir_lowering=False)
    x_in = nc.declare_dram_parameter("x", [BPC, N, DT], F32, isOutput=False)
    w_in = nc.declare_dram_parameter("w", [DT, 1, NO], F32, isOutput=False)
    cb_in = nc.declare_dram_parameter("cb", [NO], F32, isOutput=False)
    bb_in = nc.declare_dram_parameter("bb", [K, 1, N], F32, isOutput=False)
    out_d = nc.declare_dram_parameter("out", [BPC, K], F32, isOutput=True)

    AF = mybir.ActivationFunctionType
    OP = mybir.AluOpType
    RDT = BF16 if ROUTING_BF16 else F32

    with tile.TileContext(nc) as tc:
        with tc.tile_pool(name="const", bufs=1) as cpool:
            # ---- constants / inputs in SBUF ----
            MMDT = F32R if USE_F32R else F32
            xT4 = [cpool.tile([128, N], MMDT, name=f"xT4_{g}", tag=f"xT4_{g}") for g in range(2)]
            w4 = cpool.tile([128, NO], MMDT, name="w4", tag="w4")
            xn = [cpool.tile([128, 4 * DT], F32, name=f"xn_{b}", tag=f"xn_{b}")
                  for b in range(BPC)]
            wstage = cpool.tile([128, NO], F32, name="wstage", tag="wstage")
            ident = cpool.tile([128, 128], F32, name="ident", tag="ident")
            bbias = cpool.tile([128, N], F32, name="bbias", tag="bbias")
            smask = cpool.tile([128, NO], F32, name="smask", tag="smask")
            gmask = [cpool.tile([128, K], F32, name=f"gmask{c}", tag=f"gmask{c}") for c in range(5)]

            wflat = w_in.rearrange("t u o -> t (u o)")
            cbflat = cb_in.rearrange("(u o) -> u o", u=1)
            for j in range(4):
                nc.sync.dma_start(wstage[32 * j:32 * j + DT, :], wflat[:, :])
                nc.sync.dma_start(wstage[32 * j + DT:32 * j + DT + 1, :], cbflat[:, :])
                # B_bias rows for this j (same for both bgroups)
                nc.sync.dma_start(
                    bbias[32 * j:32 * j + K, :],
                    bb_in.rearrange("k u m -> k (u m)"),
                )
            nc.vector.tensor_copy(w4[:], wstage[:])

            # identity for PE transpose
            nc.gpsimd.memset(ident[:], 0.0)
            nc.gpsimd.affine_select(
                out=ident[:], in_=ident[:],
                compare_op=OP.not_equal, fill=1.0,
                base=0, pattern=[[-1, 128]], channel_multiplier=1,
            )
            # gmask[c][p, k] = 0.25 if class(c*128+p) == k else 0
            for c in range(5):
                nc.gpsimd.memset(gmask[c][:], 0.0)
                for half in range(2):
                    nc.gpsimd.affine_select(
                        out=gmask[c][64 * half:64 * (half + 1), :],
                        in_=gmask[c][64 * half:64 * (half + 1), :],
                        compare_op=OP.not_equal, fill=0.25,
                        base=-(2 * c + half), pattern=[[1, K]],
                        channel_multiplier=0,
                    )
            # ---- x path: contiguous loads + on-chip PE transposes ----
            # natural-layout load: xn[b][p, u*16+t] = x[b, u*128+p, t]
            for b in range(BPC):
                nc.sync.dma_start(
                    xn[b][:], x_in[b].rearrange("(u p) t -> p u t", p=128),
                )
            for g in range(2):
                # bias-fold row 32j+16 stays 1.0 everywhere we don't overwrite
                # (f32 bitcast: memset doesn't accept f32r, same 4-byte bits)
                nc.gpsimd.memset(xT4[g][:].bitcast(F32), 1.0)
            with tc.tile_pool(name="ps_xt", bufs=4, space="PSUM") as pxt:
                for g in range(2):
                    for j in range(4):
                        b = 4 * g + j
                        ps_x = pxt.tile([DT, N], F32, name="ps_x", tag="ps_x")
                        for u in range(4):
                            nc.tensor.transpose(
                                ps_x[:, u * 128:(u + 1) * 128],
                                xn[b][:, u * DT:(u + 1) * DT],
                                ident[:],
                            )
                        nc.vector.tensor_copy(
                            xT4[g][32 * j:32 * j + DT, :], ps_x[:],
                        )

            # smask rows 32j+k: 1.0 at cols [64k, 64k+64)
            nc.gpsimd.memset(smask[:], 1.0)
            for j in range(4):
                sl = smask[32 * j:32 * (j + 1), :]
                nc.gpsimd.affine_select(
                    out=sl, in_=sl, compare_op=OP.is_ge, fill=0.0,
                    base=0, pattern=[[1, NO]], channel_multiplier=-D,
                )
                nc.gpsimd.affine_select(
                    out=sl, in_=sl, compare_op=OP.is_ge, fill=0.0,
                    base=(D - 1), pattern=[[-1, NO]], channel_multiplier=D,
                )

            with tc.tile_pool(name="data", bufs=1) as dpool:
                yr_oi = [dpool.tile([128, 5 * N], RDT, name=f"yr_oi{b}", tag=f"yr_oi{b}")
                         for b in range(BPC)]
                yr_io = [dpool.tile([128, 4 * NO], RDT, name=f"yr_io{b}", tag=f"yr_io{b}")
                         for b in range(BPC)]
                usum = [dpool.tile([128, 5], F32, name=f"usum{b}", tag=f"usum{b}")
                        for b in range(BPC)]
                gmat = [dpool.tile([128, 5 * K], RDT, name=f"gmat{b}", tag=f"gmat{b}")
                        for b in range(BPC)]
                exp_sb = [dpool.tile([128, N], F32, name=f"exp{g}", tag=f"exp{g}") for g in range(2)]
                cb_sb = [dpool.tile([128, N], F32, name=f"cb{g}", tag=f"cb{g}") for g in range(2)]
                negmax = [dpool.tile([128, 1], F32, name=f"nm{g}", tag=f"nm{g}") for g in range(2)]
                zsum = [dpool.tile([128, 1], F32, name=f"z{g}", tag=f"z{g}") for g in range(2)]
                rz = [dpool.tile([128, 1], F32, name=f"rz{g}", tag=f"rz{g}") for g in range(2)]
                ebt = [dpool.tile([128, 4 * K], RDT, name=f"ebt{g}_{q}", tag=f"ebt{g}_{q}")
                       for g in range(2) for q in range(4)]
                sm_s = [dpool.tile([128, NO], F32, name=f"sm{g}", tag=f"sm{g}") for g in range(2)]
                sq_s = [dpool.tile([128, NO], F32, name=f"sq{g}", tag=f"sq{g}") for g in range(2)]
                n2 = dpool.tile([128, 2], F32, name="n2", tag="n2")
                t_a = dpool.tile([128, 2], F32, name="t_a", tag="t_a")
                t_b = dpool.tile([128, 2], F32, name="t_b", tag="t_b")
                t_c = dpool.tile([128, 2], F32, name="t_c", tag="t_c")
                t_d = dpool.tile([128, 2], F32, name="t_d", tag="t_d")

                evict_i = 0  # alternate DVE / ACT for PSUM evictions

                # conv (both orientations) + logits + softmax share one
                # PSUM scope (3 + 4 + 1 = 8 banks) so routing for bgroup 0
                # overlaps conv for bgroup 1 instead of waiting for pool close.
                with tc.tile_pool(name="ps_oi", bufs=4, space="PSUM") as poi, \
                     tc.tile_pool(name="ps_io", bufs=2, space="PSUM") as pio:
                    for g in range(2):
                        for c in range(5):
                            for j in range(4):
                                b = 4 * g + j
                                ps = poi.tile([128, N], F32, name="ps_oi", tag="ps_oi")
                                nc.tensor.matmul(
                                    ps[:],
                                    w4[32 * j:32 * j + DT + 1,
                                             c * 128:(c + 1) * 128],
                                    xT4[g][32 * j:32 * j + DT + 1, :],
                                    start=True, stop=True,
                                    tile_position=(32 * j, 0),
                                )
                                dst = yr_oi[b][:, c * N:(c + 1) * N]
                                acc = usum[b][:, c:c + 1]
                                if evict_i % 2 == 0:
                                    nc.vector.tensor_scalar(
                                        out=dst, in0=ps[:],
                                        scalar1=0.0, scalar2=0.0,
                                        op0=OP.max, op1=OP.add,
                                        accum_out=acc,
                                    )
                                else:
                                    nc.scalar.activation(
                                        out=dst, in_=ps[:], func=AF.Relu,
                                        accum_out=acc,
                                    )
                                evict_i += 1
                                # G for this (b, c) as soon as usum lands
                                nc.vector.tensor_scalar(
                                    out=gmat[b][:, c * K:(c + 1) * K],
                                    in0=gmask[c][:],
                                    scalar1=usum[b][:, c:c + 1], scalar2=None,
                                    op0=OP.mult,
                                )
                        for q in range(4):
                            for j in range(4):
                                b = 4 * g + j
                                ps = pio.tile([128, NO], F32, name="ps_io", tag="ps_io")
                                for s, (o0, o1) in enumerate(((0, 512), (512, NO))):
                                    nc.tensor.matmul(
                                        ps[:, o0:o1],
                                        xT4[g][32 * j:32 * j + DT + 1,
                                                     q * 128:(q + 1) * 128],
                                        w4[32 * j:32 * j + DT + 1, o0:o1],
                                        start=True, stop=True,
                                        tile_position=(32 * j, 0),
                                    )
                                dst = yr_io[b][:, q * NO:(q + 1) * NO]
                                if evict_i % 2 == 0:
                                    nc.vector.tensor_scalar(
                                        out=dst, in0=ps[:],
                                        scalar1=0.0, scalar2=None, op0=OP.max,
                                    )
                                else:
                                    nc.scalar.activation(
                                        out=dst, in_=ps[:], func=AF.Relu,
                                    )
                                evict_i += 1
                with tc.tile_pool(name="ps_l", bufs=2, space="PSUM") as pl, \
                     tc.tile_pool(name="ps_t", bufs=2, space="PSUM") as pt, \
                     tc.tile_pool(name="ps_s", bufs=2, space="PSUM") as psp:
                    # logits + softmax per bgroup (col-tiled over j)
                    for g in range(2):
                            ps_lg = pl.tile([128, N], F32, name="ps_l", tag="ps_l")
                            for c in range(5):
                                for j in range(4):
                                    b = 4 * g + j
                                    nc.tensor.matmul(
                                        ps_lg[32 * j:32 * j + K, :],
                                        gmat[b][:, c * K:(c + 1) * K],
                                        yr_oi[b][:, c * N:(c + 1) * N],
                                        start=(c == 0), stop=(c == 4),
                                        tile_position=(0, 32 * j),
                                    )
                            nc.vector.tensor_reduce(
                                out=negmax[g][:], in_=ps_lg[:],
                                op=OP.max, axis=mybir.AxisListType.X, negate=True,
                            )
                            nc.scalar.activation(
                                out=exp_sb[g][:], in_=ps_lg[:], func=AF.Exp,
                                bias=negmax[g][:], scale=1.0,
                                accum_out=zsum[g][:],
                            )
                            nc.vector.reciprocal(rz[g][:], zsum[g][:])
                            # Cb = exp/Z + B_bias   (garbage rows stay garbage)
                            nc.vector.scalar_tensor_tensor(
                                out=cb_sb[g][:], in0=exp_sb[g][:],
                                scalar=rz[g][:], in1=bbias[:],
                                op0=OP.mult, op1=OP.add,
                            )
                    # ======== phase 6: transpose Cb -> EBt tiles
                    for g in range(2):
                        for q in range(4):
                            tr = pt.tile([128, 128], F32, name="ps_tr", tag="ps_tr")
                            nc.tensor.transpose(
                                tr[:], cb_sb[g][:, q * 128:(q + 1) * 128],
                                ident[:],
                            )
                            src = tr[:].rearrange("m (a k) -> m a k", a=4)[
                                :, :, 0:K]
                            dst = ebt[4 * g + q][:].rearrange(
                                "m (a k) -> m a k", a=4)
                            if evict_i % 2 == 0:
                                nc.vector.tensor_copy(dst, src)
                            else:
                                nc.scalar.copy(dst, src)
                            evict_i += 1
                    # ======== phase 7: S = sum_q CbT_q^T @ yr_io_q  (col-tiled)
                    ps_s = [psp.tile([128, NO], F32, name="ps_s", tag="ps_s") for _ in range(2)]
                    for g in range(2):
                        for q in range(4):
                            for j in range(4):
                                b = 4 * g + j
                                for (o0, o1) in ((0, 512), (512, NO)):
                                    nc.tensor.matmul(
                                        ps_s[g][32 * j:32 * j + K, o0:o1],
                                        ebt[4 * g + q][:, j * K:(j + 1) * K],
                                        yr_io[b][:, q * NO + o0:q * NO + o1],
                                        start=(q == 0), stop=(q == 3),
                                        tile_position=(0, 32 * j),
                                    )
                    # ======== phase 8: masked norms + squash tail
                    for g in range(2):
                        nc.vector.tensor_tensor(
                            out=sm_s[g][:], in0=ps_s[g][:], in1=smask[:],
                            op=OP.mult,
                        )
                        nc.vector.scalar_tensor_tensor(
                            out=sq_s[g][:], in0=sm_s[g][:],
                            scalar=1.0, in1=sm_s[g][:],
                            op0=OP.mult, op1=OP.mult,
                            accum_out=n2[:, g:g + 1],
                        )
                    # sqrt(n2) = exp(0.5 * ln(n2)) -- stays in one table set
                    nc.scalar.activation(out=t_a[:], in_=n2[:], func=AF.Ln)
                    nc.scalar.activation(out=t_b[:], in_=t_a[:], func=AF.Exp,
                                         scale=0.5)
                    # out = (n2*sqrt) / ((n2+1)*(sqrt+eps))
                    nc.vector.tensor_scalar(out=t_c[:], in0=n2[:],
                                            scalar1=1.0, scalar2=None, op0=OP.add)
                    nc.vector.tensor_scalar(out=t_d[:], in0=t_b[:],
                                            scalar1=EPS, scalar2=None, op0=OP.add)
                    nc.vector.tensor_tensor(out=t_c[:], in0=t_c[:], in1=t_d[:],
                                            op=OP.mult)
                    nc.vector.reciprocal(t_d[:], t_c[:])
                    nc.vector.tensor_tensor(out=t_a[:], in0=n2[:], in1=t_b[:],
                                            op=OP.mult)
                    nc.vector.tensor_tensor(out=t_b[:], in0=t_a[:], in1=t_d[:],
                                            op=OP.mult)
                    # ======== output DMA: rows 32j..32j+10, col g -> out[4g+j]
                    for g in range(2):
                        for j in range(4):
                            nc.sync.dma_start(
                                out_d[4 * g + j:4 * g + j + 1, :],
                                t_b[32 * j:32 * j + K, g:g + 1],
                            )
    nc.compile()
    return nc


_PROGRAM_CACHE = None


def _get_program():
    global _PROGRAM_CACHE
    if _PROGRAM_CACHE is None:
        _PROGRAM_CACHE = _build_program()
    return _PROGRAM_CACHE


def kernel(timecaps, conv_w, conv_b, B_bias):
    timecaps = np.ascontiguousarray(np.asarray(timecaps, dtype=np.float32))
    conv_w = np.ascontiguousarray(np.asarray(conv_w, dtype=np.float32))
    conv_b = np.ascontiguousarray(np.asarray(conv_b, dtype=np.float32))
    B_bias = np.ascontiguousarray(np.asarray(B_bias, dtype=np.float32))

    nc = _get_program()
    in_maps = [
        {
            "x": timecaps[core * BPC:(core + 1) * BPC],
            "w": conv_w,
            "cb": conv_b,
            "bb": B_bias,
        }
        for core in range(NCORES)
    ]
    res = run_bass_kernel_spmd(nc, in_maps, list(range(NCORES)))
    out = np.concatenate([res.results[i]["out"] for i in range(NCORES)], axis=0)
    return out.reshape(B_FULL, K, 1).astype(np.float32)


if __name__ == "__main__":
    rng = np.random.default_rng(0)
    ins = {
        "timecaps": rng.standard_normal((B_FULL, N, DT), dtype=np.float32),
        "conv_w": (rng.standard_normal((DT, 1, NO), dtype=np.float32) * 0.05),
        "conv_b": np.zeros((NO,), dtype=np.float32),
        "B_bias": (rng.standard_normal((K, 1, N), dtype=np.float32) * 0.05),
    }
    print(kernel(**ins)[:2, :, 0])



# revision 10
# speedup vs baseline: 1.3987x; 1.3987x over previous
"""Trainium2 Bass kernel for nn_Classifier (capsule-style conv + routing).

Math (vs the jax reference):
  W = conv_w[:,0,:]                                   # [16, 640]
  y[b,i,o]   = relu(sum_t x[b,i,t] W[t,o] + conv_b[o])          (conv as matmul, K=16)
  U[b,k,i,d] = y[b,i,k*64+d]
  Usum[b,k,d]= sum_i U[b,k,i,d]
  logits     = (U . Usum)/4            -> stable softmax over i  -> C
  Cb         = C + B_bias[k,i]
  S[b,k,:]   = sum_i Cb[b,k,i] U[b,k,i,:]
  out[b,k]   = (n2/(n2+1)) * (sqrt(n2)/(sqrt(n2)+1e-7)),  n2 = |S|^2

Sharding: data-parallel over batch, 8 batches per core, 8 cores (SPMD).

v2 design (per core; b = 4g+j, g in {0,1} bgroups, j in 0..3):
  - host pre-packs inputs:
      x_img[g][32j+t, i] = x[4g+j, i, t], row 32j+16 = 1.0 (bias fold)
      w_img[32j+t, o] = W[t, o], row 32j+16 = conv_b
      bb_img[32j+k, m] = B_bias[k, m]
    so there are only 4 input DMA launches and NO on-chip x transposes.
  - conv (single orientation): per (g,c): 4 row-tiled concurrent PE matmuls
    [17,128,512] f32r; PSUM evicted with relu straight to bf16 yr_oi[b]
    ([o,i] layout) on DVE/ACT with accum_out -> usum (fp32).
  - the [i,o] orientation is NOT recomputed/evicted: one X-bar DMA transpose
    per batch (bf16, SBUF->SBUF) turns yr_oi [128, 2560] into
    yr_io [128, 20, 128] = [i%128, (c,q), o%128].
  - logits via block-diag G trick (gmat = gmask * usum, computed on GpSimd),
    softmax with exp(bias=-max, accum=Z); Cb = exp/Z + B (GpSimd STT, bf16).
  - CbT for the S matmul also via X-bar DMA transpose (no PE transposes).
  - S = sum_q CbT_q^T @ yr_io_q (col-tiled); masked square-accum -> n2;
    squash tail with sqrt computed as exp(0.5*ln(x)).
  - single batched output DMA, out dram layout [j, k, g].
"""

import numpy as np

import concourse.bass as bass
import concourse.mybir as mybir
import concourse.tile as tile
from concourse import bacc
from concourse.bass_utils import run_bass_kernel_spmd

F32 = mybir.dt.float32
F32R = mybir.dt.float32r
BF16 = mybir.dt.bfloat16

B_FULL = 64
N = 512          # num timecaps (routing dim m/i)
DT = 16          # dim timecaps (conv contraction)
K = 10           # classes
D = 64           # dim classes
NO = K * D       # 640 conv output channels
NCORES = 8
BPC = B_FULL // NCORES   # 8 batches per core
EPS = 1e-7

# which of the 40 oi evictions go to ACT (rest on DVE); ACT ops are ~1.4x
# slower per element here, so give it a ~16/40 share, spread out.
_ACT_SHARE_NUM, _ACT_SHARE_DEN = 2, 5


def _build_program():
    nc = bacc.Bacc("TRN2", target_bir_lowering=False)
    x_in = nc.declare_dram_parameter("x", [2, 128, N], F32, isOutput=False)
    w_in = nc.declare_dram_parameter("w", [128, NO], F32, isOutput=False)
    bb_in = nc.declare_dram_parameter("bb", [128, N], F32, isOutput=False)
    out_d = nc.declare_dram_parameter("out", [128, 2], F32, isOutput=True)

    AF = mybir.ActivationFunctionType
    OP = mybir.AluOpType

    with tile.TileContext(nc) as tc:
        with tc.tile_pool(name="const", bufs=1) as cpool:
            xT4 = [cpool.tile([128, N], F32, name=f"xT4_{g}", tag=f"xT4_{g}")
                   for g in range(2)]
            w4 = cpool.tile([128, NO], F32, name="w4", tag="w4")
            bbias = cpool.tile([128, N], F32, name="bbias", tag="bbias")
            gmask = cpool.tile([128, 5 * K], F32, name="gmask", tag="gmask")
            smask = cpool.tile([128, NO], F32, name="smask", tag="smask")

            for g in range(2):
                nc.sync.dma_start(xT4[g][:], x_in[g])
            nc.sync.dma_start(w4[:], w_in[:, :])
            nc.sync.dma_start(bbias[:], bb_in[:, :])

            # gmask[64h:64h+64, c*K+k] = 0.25 iff class(c*128+64h+p) == k
            nc.gpsimd.memset(gmask[:], 0.0)
            for c in range(5):
                for half in range(2):
                    sl = gmask[64 * half:64 * (half + 1), c * K:(c + 1) * K]
                    nc.gpsimd.affine_select(
                        out=sl, in_=sl, compare_op=OP.not_equal, fill=0.25,
                        base=-(2 * c + half), pattern=[[1, K]],
                        channel_multiplier=0,
                    )
            # smask rows 32j+k: 1.0 at cols [64k, 64k+64)
            nc.gpsimd.memset(smask[:], 1.0)
            for j in range(4):
                sl = smask[32 * j:32 * (j + 1), :]
                nc.gpsimd.affine_select(
                    out=sl, in_=sl, compare_op=OP.is_ge, fill=0.0,
                    base=0, pattern=[[1, NO]], channel_multiplier=-D,
                )
                nc.gpsimd.affine_select(
                    out=sl, in_=sl, compare_op=OP.is_ge, fill=0.0,
                    base=(D - 1), pattern=[[-1, NO]], channel_multiplier=D,
                )

            with tc.tile_pool(name="data", bufs=1) as dpool:
                yr_oi = [dpool.tile([128, 5 * N], BF16, name=f"yr_oi{b}", tag=f"yr_oi{b}")
                         for b in range(BPC)]
                # yr_io[b][p, (c,q,r)] = U[b, i=q*128+p, o=c*128+r]
                yr_io = [dpool.tile([128, 5 * N], BF16, name=f"yr_io{b}", tag=f"yr_io{b}")
                         for b in range(BPC)]
                usum = [dpool.tile([128, 5], F32, name=f"usum{b}", tag=f"usum{b}")
                        for b in range(BPC)]
                gmat = [dpool.tile([128, 5 * K], BF16, name=f"gmat{b}", tag=f"gmat{b}")
                        for b in range(BPC)]
                exp_sb = [dpool.tile([128, N], F32, name=f"exp{g}", tag=f"exp{g}") for g in range(2)]
                cb_sb = [dpool.tile([128, N], BF16, name=f"cb{g}", tag=f"cb{g}") for g in range(2)]
                # cbT[g][p, (q,r)] = Cb[row r, m=q*128+p]
                cbT = [dpool.tile([128, N], BF16, name=f"cbT{g}", tag=f"cbT{g}") for g in range(2)]
                negmax = [dpool.tile([128, 1], F32, name=f"nm{g}", tag=f"nm{g}") for g in range(2)]
                zsum = [dpool.tile([128, 1], F32, name=f"z{g}", tag=f"z{g}") for g in range(2)]
                rz = [dpool.tile([128, 1], F32, name=f"rz{g}", tag=f"rz{g}") for g in range(2)]
                sm_s = [dpool.tile([128, NO], BF16, name=f"sm{g}", tag=f"sm{g}") for g in range(2)]
                sq_s = dpool.tile([128, NO], BF16, name="sq_s", tag="sq_s")
                n2 = dpool.tile([128, 2], F32, name="n2", tag="n2")
                t_a = dpool.tile([128, 2], F32, name="t_a", tag="t_a")
                t_b = dpool.tile([128, 2], F32, name="t_b", tag="t_b")
                t_c = dpool.tile([128, 2], F32, name="t_c", tag="t_c")
                t_d = dpool.tile([128, 2], F32, name="t_d", tag="t_d")

                # ---- conv (single orientation) + relu-evict + transposes ----
                ei = 0
                with tc.tile_pool(name="ps_oi", bufs=4, space="PSUM") as poi:
                    for g in range(2):
                        for c in range(5):
                            for j in range(4):
                                b = 4 * g + j
                                ps = poi.tile([128, N], F32, name="ps_oi", tag="ps_oi")
                                nc.tensor.matmul(
                                    ps[:],
                                    w4[32 * j:32 * j + DT + 1,
                                       c * 128:(c + 1) * 128],
                                    xT4[g][32 * j:32 * j + DT + 1, :],
                                    start=True, stop=True,
                                    tile_position=(32 * j, 0),
                                )
                                dst = yr_oi[b][:, c * N:(c + 1) * N]
                                acc = usum[b][:, c:c + 1]
                                on_act = (ei % 2 == 0)
                                if on_act:
                                    nc.scalar.activation(
                                        out=dst, in_=ps[:], func=AF.Relu,
                                        accum_out=acc,
                                    )
                                else:
                                    nc.vector.tensor_scalar(
                                        out=dst, in0=ps[:],
                                        scalar1=0.0, scalar2=0.0,
                                        op0=OP.max, op1=OP.add,
                                        accum_out=acc,
                                    )
                                ei += 1
                        for j in range(4):
                            b = 4 * g + j
                            # G for batch b once all usum chunks land
                            nc.vector.tensor_tensor(
                                out=gmat[b][:].rearrange("p (c k) -> p c k", c=5),
                                in0=gmask[:].rearrange("p (c k) -> p c k", c=5),
                                in1=usum[b][:, :, None].broadcast_to(
                                    (128, 5, K)),
                                op=OP.mult,
                            )
                        for j in range(4):
                            b = 4 * g + j
                            nc.sync.dma_start_transpose(
                                out=yr_io[b][:].rearrange("p (d r) -> p d r", r=128),
                                in_=yr_oi[b][:],
                            )

                # ---- routing: logits + softmax + Cb(+transpose) + S + tail ----
                with tc.tile_pool(name="ps_l", bufs=2, space="PSUM") as pl, \
                     tc.tile_pool(name="ps_s", bufs=1, space="PSUM") as psp:
                    for g in range(2):
                        ps_lg = pl.tile([128, N], F32, name="ps_l", tag="ps_l")
                        for c in range(5):
                            for j in range(4):
                                b = 4 * g + j
                                nc.tensor.matmul(
                                    ps_lg[32 * j:32 * j + K, :],
                                    gmat[b][:, c * K:(c + 1) * K],
                                    yr_oi[b][:, c * N:(c + 1) * N],
                                    start=(c == 0), stop=(c == 4),
                                    tile_position=(0, 32 * j),
                                )
                        nc.vector.tensor_reduce(
                            out=negmax[g][:], in_=ps_lg[:],
                            op=OP.max, axis=mybir.AxisListType.X, negate=True,
                        )
                        nc.scalar.activation(
                            out=exp_sb[g][:], in_=ps_lg[:], func=AF.Exp,
                            bias=negmax[g][:], scale=1.0,
                            accum_out=zsum[g][:],
                        )
                        nc.vector.reciprocal(rz[g][:], zsum[g][:])
                        # Cb = exp/Z + B_bias   (garbage rows stay garbage)
                        nc.vector.scalar_tensor_tensor(
                            out=cb_sb[g][:], in0=exp_sb[g][:],
                            scalar=rz[g][:], in1=bbias[:],
                            op0=OP.mult, op1=OP.add,
                        )
                        # CbT via X-bar (launched from the ACT queue so the
                        # sync queue's yr transposes aren't blocked behind it)
                        nc.scalar.dma_start_transpose(
                            out=cbT[g][:].rearrange("p (d r) -> p d r", r=128),
                            in_=cb_sb[g][:],
                        )
                    # S = sum_q CbT_q^T @ yr_io_q  (col-tiled over j)
                    for g in range(2):
                        ps_s = psp.tile([128, NO], F32, name="ps_s", tag="ps_s")
                        cbT_v = cbT[g][:].rearrange("p (q r) -> p q r", q=4)
                        for q in range(4):
                            for j in range(4):
                                b = 4 * g + j
                                io_v = yr_io[b][:].rearrange(
                                    "p (c q2 r) -> p c q2 r", c=5, q2=4)
                                nc.tensor.matmul(
                                    ps_s[32 * j:32 * j + K, 0:512],
                                    cbT_v[:, q, 32 * j:32 * j + K],
                                    io_v[:, 0:4, q, :],
                                    start=(q == 0), stop=(q == 3),
                                    tile_position=(0, 32 * j),
                                )
                                nc.tensor.matmul(
                                    ps_s[32 * j:32 * j + K, 512:NO],
                                    cbT_v[:, q, 32 * j:32 * j + K],
                                    io_v[:, 4, q, :],
                                    start=(q == 0), stop=(q == 3),
                                    tile_position=(0, 32 * j),
                                )
                        nc.vector.tensor_tensor(
                            out=sm_s[g][:], in0=ps_s[:], in1=smask[:],
                            op=OP.mult,
                        )
                        nc.vector.scalar_tensor_tensor(
                            out=sq_s[:], in0=sm_s[g][:],
                            scalar=1.0, in1=sm_s[g][:],
                            op0=OP.mult, op1=OP.mult,
                            accum_out=n2[:, g:g + 1],
                        )
                    # sqrt(n2) = exp(0.5 * ln(n2)) -- stays in one table set
                    nc.scalar.activation(out=t_a[:], in_=n2[:], func=AF.Ln)
                    nc.scalar.activation(out=t_b[:], in_=t_a[:], func=AF.Exp,
                                         scale=0.5)
                    # out = (n2*sqrt) / ((n2+1)*(sqrt+eps))
                    nc.vector.tensor_scalar(out=t_c[:], in0=n2[:],
                                            scalar1=1.0, scalar2=None, op0=OP.add)
                    nc.vector.tensor_scalar(out=t_d[:], in0=t_b[:],
                                            scalar1=EPS, scalar2=None, op0=OP.add)
                    nc.vector.tensor_tensor(out=t_c[:], in0=t_c[:], in1=t_d[:],
                                            op=OP.mult)
                    nc.vector.reciprocal(t_d[:], t_c[:])
                    nc.vector.tensor_tensor(out=t_a[:], in0=n2[:], in1=t_b[:],
                                            op=OP.mult)
                    nc.vector.tensor_tensor(out=t_b[:], in0=t_a[:], in1=t_d[:],
                                            op=OP.mult)
                    # single batched output DMA of the whole [128,2] tile;
                    # the host picks rows 32j+k (b=4g+j at column g).
                    nc.sync.dma_start(out_d[:, :], t_b[:])
    nc.compile()
    return nc


_PROGRAM_CACHE = None


def _get_program():
    global _PROGRAM_CACHE
    if _PROGRAM_CACHE is None:
        _PROGRAM_CACHE = _build_program()
    return _PROGRAM_CACHE


def _make_in_maps(timecaps, conv_w, conv_b, B_bias):
    x = np.ascontiguousarray(np.asarray(timecaps, dtype=np.float32))
    W = np.asarray(conv_w, dtype=np.float32)[:, 0, :]          # [16, 640]
    cb = np.asarray(conv_b, dtype=np.float32)                  # [640]
    bb = np.asarray(B_bias, dtype=np.float32)[:, 0, :]         # [10, 512]

    w_img = np.zeros((128, NO), dtype=np.float32)
    bb_img = np.zeros((128, N), dtype=np.float32)
    for j in range(4):
        w_img[32 * j:32 * j + DT] = W
        w_img[32 * j + DT] = cb
        bb_img[32 * j:32 * j + K] = bb

    in_maps = []
    for core in range(NCORES):
        xc = x[core * BPC:(core + 1) * BPC]                    # [8, 512, 16]
        x_img = np.zeros((2, 4, 32, N), dtype=np.float32)
        x_img[:, :, :DT, :] = xc.reshape(2, 4, N, DT).transpose(0, 1, 3, 2)
        x_img[:, :, DT, :] = 1.0
        in_maps.append({
            "x": np.ascontiguousarray(x_img.reshape(2, 128, N)),
            "w": w_img,
            "bb": bb_img,
        })
    return in_maps


def kernel(timecaps, conv_w, conv_b, B_bias):
    nc = _get_program()
    in_maps = _make_in_maps(timecaps, conv_w, conv_b, B_bias)
    res = run_bass_kernel_spmd(nc, in_maps, list(range(NCORES)))
    outs = []
    for c in range(NCORES):
        r = res.results[c]["out"]                              # [128, 2]
        rr = r.reshape(4, 32, 2)[:, 0:K, :]                    # [j, k, g]
        outs.append(np.transpose(rr, (2, 0, 1)).reshape(BPC, K))
    return np.concatenate(outs, axis=0).reshape(B_FULL, K, 1).astype(np.float32)


if __name__ == "__main__":
    rng = np.random.default_rng(0)
    ins = {
        "timecaps": rng.standard_normal((B_FULL, N, DT), dtype=np.float32),
        "conv_w": (rng.standard_normal((DT, 1, NO), dtype=np.float32) * 0.05),
        "conv_b": np.zeros((NO,), dtype=np.float32),
        "B_bias": (rng.standard_normal((K, 1, N), dtype=np.float32) * 0.05),
    }
    print(kernel(**ins)[:2, :, 0])
